# revision 11
# baseline (speedup 1.0000x reference)
"""Trainium2 Bass kernel for a 4-layer LSTM classifier (H=16) over 8 NeuronCores.

Strategy: pure data parallel, batch 256 -> 32/core. Per core:
  phase 1: input projection pre0 = x @ W_ih_l0a^T streamed from HBM; host
           pre-transposes x so contraction dim I lands on SBUF partitions
           with fully contiguous DMA rows. Output is partition-regrouped
           via SBUF DMA into a [16, (type, b, tl)] layout.
  phase 2: wavefront recurrence over (layer, t): all 4 layers' gates via 4
           matmuls (one per gate type i/f/o/g) into one [64, 128] PSUM tile
           (col = 32*type + b). All elementwise ops are partition-aligned
           [64, *] ops; h writes land directly in h_all[0:64].
  phase 3: FC(16->16)+ReLU via select-folded matmul on h_all, FC(16->15),
           softmax, DMA out [32, 15] per core.
"""

import sys

if "/opt/trn_rl_repo" not in sys.path:
    sys.path.insert(0, "/opt/trn_rl_repo")

import numpy as np

# ---- problem constants (hardcoded per contract) ----
B, T, I, H, C = 256, 200, 1086, 16, 15
NCORES = 8
BL = B // NCORES          # 32 batch per core
TL = 8                    # t-interleave factor
THI = T // TL             # 25
NCOLS = BL * T            # 6400
CHUNK = 512               # phase-1 matmul free dim (= 2 t_hi blocks = 16 t)
NCHUNK = (NCOLS + CHUNK - 1) // CHUNK  # 13 (last = 256)
KCH = [128] * 8 + [62]    # 1086 contraction chunks
NSTEP = T + 3             # 203 wavefront steps

CFG = dict(
    x_dtype="float32",     # or "bfloat16"
)

_BUILD_CACHE = {}


def _np_dt(name):
    import ml_dtypes
    return np.dtype(ml_dtypes.bfloat16) if name == "bfloat16" else np.dtype(name)


def _gate_rows(w):
    # torch gate row order in 4H matrices: i, f, g, o
    return dict(i=w[0:H], f=w[H:2 * H], g=w[2 * H:3 * H], o=w[3 * H:4 * H])


TYPES = ["i", "f", "o", "g"]  # gate-type order used everywhere on-chip


def build_host_constants(wd, x_dtype):
    f32 = np.float32
    # phase-1 W: rows I, cols 64 = (type-major: i0,f0,o0,g0) x16 units
    g0 = _gate_rows(wd["w_ih_l0a"])
    W_proj = np.zeros((I, 64), f32)
    for j, t in enumerate(TYPES):
        W_proj[:, 16 * j:16 * j + 16] = g0[t].T
    W_proj = W_proj.astype(_np_dt(x_dtype))

    # recurrence weights: per gate type, lhsT [65, 64]
    # h_all rows: h0 0:16, h1 16:32, h2 32:48, h3 48:64, ONE 64
    # cols: unit m = 16*l + u
    hh = [_gate_rows(wd["w_hh_l0a"]), _gate_rows(wd["w_hh_l0b"]),
          _gate_rows(wd["w_hh_l1a"]), _gate_rows(wd["w_hh_l1b"])]
    ih = [None, _gate_rows(wd["w_ih_l0b"]), _gate_rows(wd["w_ih_l1a"]),
          _gate_rows(wd["w_ih_l1b"])]
    bb = [_gate_rows(wd["b_l0a"][:, None]), _gate_rows(wd["b_l0b"][:, None]),
          _gate_rows(wd["b_l1a"][:, None]), _gate_rows(wd["b_l1b"][:, None])]
    lhsT = {}
    for t in TYPES:
        M = np.zeros((65, 64), f32)
        for l in range(4):
            cs = slice(16 * l, 16 * l + 16)
            M[16 * l:16 * l + 16, cs] = hh[l][t].T      # recurrent h_l
            if l >= 1:
                M[16 * (l - 1):16 * l, cs] = ih[l][t].T  # input h_{l-1}
            M[64, cs] = bb[l][t][:, 0]                   # bias
        lhsT[t] = M

    # fc1 folded onto h_all: out1[u,b] = sum_k W1e[k,u] h_all[k,b]
    W1e = np.zeros((65, 16), f32)
    W1e[48:64] = wd["w_fc1"].T      # h3 rows
    W1e[64] = wd["b_fc1"]
    # relu2 tile is [33, BL]: rows 0:16 = relu(fc1), rows 16:32 = zeros,
    # row 32 = ones (32-aligned partition for the memset)
    W2 = np.zeros((33, 15), f32)
    W2[0:16] = wd["w_fc2"].T
    W2[32] = wd["b_fc2"]
    return dict(W_proj=W_proj, lhsT_i=lhsT["i"], lhsT_f=lhsT["f"],
                lhsT_o=lhsT["o"], lhsT_g=lhsT["g"], W1e=W1e, W2=W2)


def build_bass(x_dtype="float32"):
    from concourse import bacc, mybir

    from concourse.tile import TileContext

    dt = mybir.dt
    xdt = dt.bfloat16 if x_dtype == "bfloat16" else dt.float32
    f32 = dt.float32
    AF = mybir.ActivationFunctionType
    ALU = mybir.AluOpType

    nc = bacc.Bacc("TRN2", target_bir_lowering=False, debug=False,
                   num_devices=NCORES)

    xin = nc.dram_tensor("x", [I, NCOLS], xdt, kind="ExternalInput").ap()
    wproj_d = nc.dram_tensor("wproj", [I, 64], xdt, kind="ExternalInput").ap()
    lhs_d = {t: nc.dram_tensor(f"lhs_{t}", [65, 64], f32,
                               kind="ExternalInput").ap() for t in TYPES}
    w1_d = nc.dram_tensor("w1", [65, 16], f32, kind="ExternalInput").ap()
    w2_d = nc.dram_tensor("w2", [33, 15], f32, kind="ExternalInput").ap()
    out_d = nc.dram_tensor("out", [BL, C], f32, kind="ExternalOutput").ap()

    with TileContext(nc) as tc:
        import contextlib
        with contextlib.ExitStack() as ctx:
            wpool = ctx.enter_context(tc.tile_pool(name="weights", bufs=9))
            xpool = ctx.enter_context(tc.tile_pool(name="xtiles", bufs=4))
            prepool = ctx.enter_context(tc.tile_pool(name="pre", bufs=THI))
            state = ctx.enter_context(tc.tile_pool(name="state", bufs=1))
            work = ctx.enter_context(tc.tile_pool(name="work", bufs=3))
            pg_pool = ctx.enter_context(
                tc.tile_pool(name="pgates", bufs=3, space="PSUM"))
            px_pool = ctx.enter_context(
                tc.tile_pool(name="pproj", bufs=2, space="PSUM"))
            pfc_pool = ctx.enter_context(
                tc.tile_pool(name="pfc", bufs=1, space="PSUM"))

            # --- weights ---
            wproj_t = []
            k0 = 0
            for kk in KCH:
                wt = wpool.tile([128, 64], xdt, tag="wproj")
                nc.sync.dma_start(out=wt[0:kk, :], in_=wproj_d[k0:k0 + kk, :])
                wproj_t.append(wt)
                k0 += kk
            lhs = {}
            for t in TYPES:
                lt = wpool.tile([65, 64], f32, tag=f"lhs_{t}")
                nc.sync.dma_start(out=lt[:], in_=lhs_d[t][:])
                lhs[t] = lt
            w1 = wpool.tile([65, 16], f32, tag="w1")
            nc.sync.dma_start(out=w1[:], in_=w1_d[:])
            w2 = wpool.tile([33, 15], f32, tag="w2")
            nc.sync.dma_start(out=w2[:], in_=w2_d[:])

            # --- persistent state ---
            h_all = state.tile([65, BL], f32, tag="h_all")
            nc.vector.memset(h_all[:], 0.0)
            nc.vector.memset(h_all[64:65, :], 1.0)
            # gct: cols 0:BL = tanh(g_raw), cols BL:2BL = c (persistent)
            gct = state.tile([64, 2 * BL], f32, tag="gct")
            nc.vector.memset(gct[:], 0.0)
            relu2 = state.tile([33, BL], f32, tag="relu2")
            nc.vector.memset(relu2[:], 0.0)
            nc.vector.memset(relu2[32:33, :], 1.0)

            pre_tiles = [None] * THI

            def emit_phase1_chunk(ci):
                c0 = ci * CHUNK
                cw = min(CHUNK, NCOLS - c0)
                px = px_pool.tile([64, CHUNK], f32, tag="px")
                k0 = 0
                for ki, kk in enumerate(KCH):
                    xt = xpool.tile([128, CHUNK], xdt, tag="xt")
                    nc.sync.dma_start(out=xt[0:kk, 0:cw],
                                      in_=xin[k0:k0 + kk, c0:c0 + cw])
                    nc.tensor.matmul(px[:, 0:cw], wproj_t[ki][0:kk, :],
                                     xt[0:kk, 0:cw],
                                     start=(ki == 0), stop=(ki == len(KCH) - 1))
                    k0 += kk
                # stage psum -> SBUF, then partition-regroup into pre tiles
                # [16, (type, b, tl)] via SBUF->SBUF DMA
                stage = xpool.tile([64, CHUNK], f32, tag="stage")
                nc.scalar.copy(stage[:, 0:cw], px[:, 0:cw])
                nblk = cw // (BL * TL)
                for bi in range(nblk):
                    th = (c0 // (BL * TL)) + bi
                    pt = prepool.tile([16, 4, BL, TL], f32, tag="pre")
                    for j in range(4):
                        src = stage[16 * j:16 * j + 16,
                                    bi * BL * TL:(bi + 1) * BL * TL]
                        nc.sync.dma_start(out=pt[:, j, :, :], in_=src)
                    pre_tiles[th] = pt

            def emit_step(s):
                lmin = max(0, s - (T - 1))
                lmax = min(3, s)
                # write range for state updates; starts must be 32-aligned,
                # so widen r0 down (clobbered rows are only read by inactive
                # layers afterwards -- harmless garbage)
                r0 = (16 * lmin // 32) * 32
                r1 = 16 * (lmax + 1)
                pg = pg_pool.tile([64, 4 * BL], f32, tag="pg")
                for j, t in enumerate(TYPES):
                    nc.tensor.matmul(pg[:, 32 * j:32 * j + 32], lhs[t][:],
                                     h_all[:], start=True, stop=True)
                if s < T:
                    th, tl = s // TL, s % TL
                    nc.vector.tensor_tensor(pg[0:16, :], pg[0:16, :],
                                            pre_tiles[th][:, :, :, tl],
                                            ALU.add)
                sifo = work.tile([64, 3 * BL], f32, tag="sifo")
                nc.scalar.activation(sifo[:], pg[:, 0:3 * BL], AF.Sigmoid)
                nc.scalar.activation(gct[:, 0:BL], pg[:, 3 * BL:4 * BL],
                                     AF.Tanh)
                tmp = work.tile([64, 2 * BL], f32, tag="tmp")
                nc.vector.tensor_tensor(tmp[:], sifo[:, 0:2 * BL], gct[:],
                                        ALU.mult)
                nc.vector.tensor_tensor(gct[r0:r1, BL:2 * BL],
                                        tmp[r0:r1, 0:BL],
                                        tmp[r0:r1, BL:2 * BL], ALU.add)
                tct = work.tile([64, BL], f32, tag="tct")
                nc.scalar.activation(tct[:], gct[:, BL:2 * BL], AF.Tanh)
                nc.vector.tensor_tensor(h_all[r0:r1, :],
                                        sifo[r0:r1, 2 * BL:3 * BL],
                                        tct[r0:r1, :], ALU.mult)

            # --- emission: interleave phase-1 chunks with recurrence ---
            steps_done = 0
            for ci in range(NCHUNK):
                emit_phase1_chunk(ci)
                tmax = min(T, (ci + 1) * (CHUNK // BL))
                while steps_done < tmax:
                    emit_step(steps_done)
                    steps_done += 1
            while steps_done < NSTEP:
                emit_step(steps_done)
                steps_done += 1

            # --- FC + softmax ---
            p1 = pfc_pool.tile([16, BL], f32, tag="p1")
            nc.tensor.matmul(p1[:], w1[:], h_all[:], start=True, stop=True)
            nc.scalar.activation(relu2[0:16, :], p1[:], AF.Relu)
            p2 = pfc_pool.tile([BL, C], f32, tag="p2")
            nc.tensor.matmul(p2[:], relu2[:], w2[:], start=True, stop=True)
            negmax = work.tile([BL, 1], f32, tag="negmax")
            nc.vector.reduce_max(negmax[:], p2[:], mybir.AxisListType.X,
                                 negate=True)
            esum = work.tile([BL, 1], f32, tag="esum")
            evals = work.tile([BL, C], f32, tag="evals")
            nc.scalar.activation(evals[:], p2[:], AF.Exp, bias=negmax[:],
                                 accum_out=esum[:])
            rinv = work.tile([BL, 1], f32, tag="rinv")
            nc.vector.reciprocal(rinv[:], esum[:])
            prob = work.tile([BL, C], f32, tag="prob")
            nc.vector.tensor_scalar(prob[:], evals[:], rinv[:], None, ALU.mult)
            nc.sync.dma_start(out=out_d[:], in_=prob[:])

    nc.compile()
    return nc


def _prep_inputs(inputs, x_dtype):
    x = inputs["x"]
    consts = build_host_constants(inputs, x_dtype)
    xdt = _np_dt(x_dtype)
    in_maps = []
    for g in range(NCORES):
        xc = x[g * BL:(g + 1) * BL]                      # [32, 200, 1086]
        xr = xc.reshape(BL, THI, TL, I).transpose(3, 1, 0, 2)  # [I,25,32,8]
        xf = np.ascontiguousarray(xr).reshape(I, NCOLS).astype(xdt)
        m = dict(x=xf, wproj=consts["W_proj"], w1=consts["W1e"],
                 w2=consts["W2"])
        for t in TYPES:
            m[f"lhs_{t}"] = consts[f"lhsT_{t}"]
        in_maps.append(m)
    return in_maps


def kernel(**inputs):
    from concourse.bass_utils import run_bass_kernel_spmd

    x_dtype = CFG["x_dtype"]
    key = ("nc", x_dtype)
    if key not in _BUILD_CACHE:
        _BUILD_CACHE[key] = build_bass(x_dtype)
    nc = _BUILD_CACHE[key]
    in_maps = _prep_inputs(inputs, x_dtype)
    res = run_bass_kernel_spmd(nc, in_maps, list(range(NCORES)))
    out = np.concatenate([res.results[g]["out"] for g in range(NCORES)], axis=0)
    return out.astype(np.float32)


# revision 22
# speedup vs baseline: 1.4515x; 1.4515x over previous
"""Trainium2 Bass kernel for a 4-layer LSTM classifier (H=16) over 8 NeuronCores.

Strategy: pure data parallel, batch 256 -> 32/core. Per core:
  phase 1: input projection pre0 = x @ W_ih_l0a^T streamed from HBM; host
           pre-transposes x so contraction dim I lands on SBUF partitions
           with fully contiguous DMA rows. Output is partition-regrouped
           via SBUF DMA into a [16, (type, b, tl)] layout.
  phase 2: wavefront recurrence over (layer, t): all 4 layers' gates via 4
           matmuls (one per gate type i/f/o/g) into one [64, 128] PSUM tile
           (col = 32*type + b). All elementwise ops are partition-aligned
           [64, *] ops; h writes land directly in h_all[0:64].
  phase 3: FC(16->16)+ReLU via select-folded matmul on h_all, FC(16->15),
           softmax, DMA out [32, 15] per core.
"""

import sys

if "/opt/trn_rl_repo" not in sys.path:
    sys.path.insert(0, "/opt/trn_rl_repo")

import numpy as np

# ---- problem constants (hardcoded per contract) ----
B, T, I, H, C = 256, 200, 1086, 16, 15
NCORES = 8
BL = B // NCORES          # 32 batch per core
TL = 8                    # t-interleave factor
THI = T // TL             # 25
NCOLS = BL * T            # 6400
CHUNK = 512               # phase-1 matmul free dim (= 2 t_hi blocks = 16 t)
NCHUNK = (NCOLS + CHUNK - 1) // CHUNK  # 13 (last = 256)
KCH = [128] * 8 + [62]    # 1086 contraction chunks
NSTEP = T + 3             # 203 wavefront steps

CFG = dict(
    x_dtype="bfloat16",    # or "float32"
    nchains=2,             # independent phase-offset recurrence chains
)

_BUILD_CACHE = {}


def _np_dt(name):
    import ml_dtypes
    return np.dtype(ml_dtypes.bfloat16) if name == "bfloat16" else np.dtype(name)


def _gate_rows(w):
    # torch gate row order in 4H matrices: i, f, g, o
    return dict(i=w[0:H], f=w[H:2 * H], g=w[2 * H:3 * H], o=w[3 * H:4 * H])


TYPES = ["i", "f", "o", "g"]  # gate-type order used everywhere on-chip


def build_host_constants(wd, x_dtype):
    f32 = np.float32
    # phase-1 W: rows I, cols 64 = (type-major: i0,f0,o0,g0) x16 units
    g0 = _gate_rows(wd["w_ih_l0a"])
    W_proj = np.zeros((I, 64), f32)
    for j, t in enumerate(TYPES):
        W_proj[:, 16 * j:16 * j + 16] = g0[t].T
    W_proj = W_proj.astype(_np_dt(x_dtype))

    # recurrence weights: per gate type, lhsT [65, 64]
    # h_all rows: h0 0:16, h1 16:32, h2 32:48, h3 48:64, ONE 64
    # cols: unit m = 16*l + u
    hh = [_gate_rows(wd["w_hh_l0a"]), _gate_rows(wd["w_hh_l0b"]),
          _gate_rows(wd["w_hh_l1a"]), _gate_rows(wd["w_hh_l1b"])]
    ih = [None, _gate_rows(wd["w_ih_l0b"]), _gate_rows(wd["w_ih_l1a"]),
          _gate_rows(wd["w_ih_l1b"])]
    bb = [_gate_rows(wd["b_l0a"][:, None]), _gate_rows(wd["b_l0b"][:, None]),
          _gate_rows(wd["b_l1a"][:, None]), _gate_rows(wd["b_l1b"][:, None])]
    lhsT = {}
    for t in TYPES:
        M = np.zeros((65, 64), f32)
        for l in range(4):
            cs = slice(16 * l, 16 * l + 16)
            M[16 * l:16 * l + 16, cs] = hh[l][t].T      # recurrent h_l
            if l >= 1:
                M[16 * (l - 1):16 * l, cs] = ih[l][t].T  # input h_{l-1}
            M[64, cs] = bb[l][t][:, 0]                   # bias
        lhsT[t] = M

    # fc1 folded onto h_all: out1[u,b] = sum_k W1e[k,u] h_all[k,b]
    W1e = np.zeros((65, 16), f32)
    W1e[48:64] = wd["w_fc1"].T      # h3 rows
    W1e[64] = wd["b_fc1"]
    # relu2 tile is [33, BL]: rows 0:16 = relu(fc1), rows 16:32 = zeros,
    # row 32 = ones (32-aligned partition for the memset)
    W2 = np.zeros((33, 15), f32)
    W2[0:16] = wd["w_fc2"].T
    W2[32] = wd["b_fc2"]
    # pre-injection select: maps pre row u -> pg row u (l0 units), zeros rows 16:64
    SEL = np.zeros((16, 64), f32)
    SEL[np.arange(16), np.arange(16)] = 1.0
    return dict(W_proj=W_proj, lhsT_i=lhsT["i"], lhsT_f=lhsT["f"],
                lhsT_o=lhsT["o"], lhsT_g=lhsT["g"], W1e=W1e, W2=W2, SEL=SEL)


def build_bass(x_dtype="float32", nchains=2):
    from concourse import bacc, mybir

    from concourse.tile import TileContext

    dt = mybir.dt
    xdt = dt.bfloat16 if x_dtype == "bfloat16" else dt.float32
    f32 = dt.float32
    AF = mybir.ActivationFunctionType
    ALU = mybir.AluOpType

    nc = bacc.Bacc("TRN2", target_bir_lowering=False, debug=False,
                   num_devices=NCORES)

    xin = nc.dram_tensor("x", [I, NCOLS], xdt, kind="ExternalInput").ap()
    wproj_d = nc.dram_tensor("wproj", [I, 64], xdt, kind="ExternalInput").ap()
    lhs_d = {t: nc.dram_tensor(f"lhs_{t}", [65, 64], f32,
                               kind="ExternalInput").ap() for t in TYPES}
    w1_d = nc.dram_tensor("w1", [65, 16], f32, kind="ExternalInput").ap()
    w2_d = nc.dram_tensor("w2", [33, 15], f32, kind="ExternalInput").ap()
    sel_d = nc.dram_tensor("sel", [16, 64], f32, kind="ExternalInput").ap()
    out_d = nc.dram_tensor("out", [BL, C], f32, kind="ExternalOutput").ap()

    with TileContext(nc) as tc:
        import contextlib
        with contextlib.ExitStack() as ctx:
            wpool = ctx.enter_context(tc.tile_pool(name="weights", bufs=9))
            xpool = ctx.enter_context(tc.tile_pool(name="xtiles", bufs=4))
            prepool = ctx.enter_context(tc.tile_pool(name="pre", bufs=THI))
            state = ctx.enter_context(tc.tile_pool(name="state", bufs=1))
            work = ctx.enter_context(tc.tile_pool(name="work", bufs=3))
            pg_pool = ctx.enter_context(
                tc.tile_pool(name="pgates", bufs=2, space="PSUM"))
            pgg_pool = ctx.enter_context(
                tc.tile_pool(name="pgg", bufs=1, space="PSUM"))
            px_pool = ctx.enter_context(
                tc.tile_pool(name="pproj", bufs=2, space="PSUM"))

            # --- weights ---
            wproj_t = []
            k0 = 0
            for kk in KCH:
                wt = wpool.tile([128, 64], xdt, tag="wproj")
                nc.sync.dma_start(out=wt[0:kk, :], in_=wproj_d[k0:k0 + kk, :])
                wproj_t.append(wt)
                k0 += kk
            lhs = {}
            for t in TYPES:
                lt = wpool.tile([65, 64], f32, tag=f"lhs_{t}")
                nc.sync.dma_start(out=lt[:], in_=lhs_d[t][:])
                lhs[t] = lt
            w1 = wpool.tile([65, 16], f32, tag="w1")
            nc.sync.dma_start(out=w1[:], in_=w1_d[:])
            w2 = wpool.tile([33, 15], f32, tag="w2")
            nc.sync.dma_start(out=w2[:], in_=w2_d[:])
            sel = wpool.tile([16, 64], f32, tag="sel")
            nc.sync.dma_start(out=sel[:], in_=sel_d[:])

            # --- persistent state (one set per chain) ---
            CH = nchains
            BW = BL // CH
            h_alls, gcts, relu2s = [], [], []
            for c in range(CH):
                h_all = state.tile([65, BW], f32, tag=f"h_all{c}")
                nc.vector.memset(h_all[:], 0.0)
                nc.vector.memset(h_all[64:65, :], 1.0)
                # gct: cols 0:BW = tanh(g_raw), cols BW:2BW = c (persistent)
                gct = state.tile([64, 2 * BW], f32, tag=f"gct{c}")
                nc.vector.memset(gct[:], 0.0)
                relu2 = state.tile([33, BW], f32, tag=f"relu2{c}")
                nc.vector.memset(relu2[:], 0.0)
                nc.vector.memset(relu2[32:33, :], 1.0)
                h_alls.append(h_all)
                gcts.append(gct)
                relu2s.append(relu2)

            pre_tiles = [None] * THI

            def emit_phase1_chunk(ci):
                c0 = ci * CHUNK
                cw = min(CHUNK, NCOLS - c0)
                px = px_pool.tile([64, CHUNK], f32, tag="px")
                k0 = 0
                for ki, kk in enumerate(KCH):
                    xt = xpool.tile([128, CHUNK], xdt, tag="xt")
                    nc.sync.dma_start(out=xt[0:kk, 0:cw],
                                      in_=xin[k0:k0 + kk, c0:c0 + cw])
                    nc.tensor.matmul(px[:, 0:cw], wproj_t[ki][0:kk, :],
                                     xt[0:kk, 0:cw],
                                     start=(ki == 0), stop=(ki == len(KCH) - 1))
                    k0 += kk
                # stage psum -> SBUF, then partition-regroup into pre tiles
                # [16, (type, b, tl)] via SBUF->SBUF DMA
                stage = xpool.tile([64, CHUNK], f32, tag="stage")
                nc.scalar.copy(stage[:, 0:cw], px[:, 0:cw])
                nblk = cw // (BL * TL)
                for bi in range(nblk):
                    th = (c0 // (BL * TL)) + bi
                    pt = prepool.tile([16, 4, BL, TL], f32, tag="pre")
                    for j in range(4):
                        src = stage[16 * j:16 * j + 16,
                                    bi * BL * TL:(bi + 1) * BL * TL]
                        nc.sync.dma_start(out=pt[:, j, :, :], in_=src)
                    pre_tiles[th] = pt

            def emit_step(s, c):
                h_all, gct = h_alls[c], gcts[c]
                lmin = max(0, s - (T - 1))
                lmax = min(3, s)
                # write range for state updates; starts must be 32-aligned,
                # so widen r0 down (clobbered rows are only read by inactive
                # layers afterwards -- harmless garbage)
                r0 = (16 * lmin // 32) * 32
                r1 = 16 * (lmax + 1)
                # g gates in their own psum tile/accum-group so TANHG can
                # start right after mm_g, hiding under the i/f/o matmuls
                pg = pg_pool.tile([64, 3 * BW], f32, tag=f"pg{c}")
                pgg = pgg_pool.tile([64, BW], f32, tag=f"pgg{c}")
                has_pre = s < T
                if has_pre:
                    th, tl = s // TL, s % TL
                    pslice = pre_tiles[th][:, :, c * BW:(c + 1) * BW, tl]
                    nc.tensor.matmul(pgg[:], sel[:], pslice[:, 3, :],
                                     start=True, stop=False,
                                     skip_group_check=True)
                    nc.tensor.matmul(pg[:], sel[:], pslice[:, 0:3, :],
                                     start=True, stop=False,
                                     skip_group_check=True)
                nc.tensor.matmul(pgg[:], lhs["g"][:], h_all[:],
                                 start=not has_pre, stop=True,
                                 skip_group_check=True)
                nc.scalar.activation(gct[:, 0:BW], pgg[:], AF.Tanh)
                for j, t in enumerate(TYPES[:3]):
                    nc.tensor.matmul(pg[:, BW * j:BW * (j + 1)], lhs[t][:],
                                     h_all[:], start=not has_pre, stop=True,
                                     skip_group_check=True)
                sifo = work.tile([64, 3 * BW], f32, tag=f"sifo{c}")
                nc.scalar.activation(sifo[:], pg[:], AF.Sigmoid)
                tmp = work.tile([64, 2 * BW], f32, tag=f"tmp{c}")
                nc.vector.tensor_tensor(tmp[:], sifo[:, 0:2 * BW], gct[:],
                                        ALU.mult)
                nc.vector.tensor_tensor(gct[r0:r1, BW:2 * BW],
                                        tmp[r0:r1, 0:BW],
                                        tmp[r0:r1, BW:2 * BW], ALU.add)
                tct = work.tile([64, BW], f32, tag=f"tct{c}")
                nc.scalar.activation(tct[:], gct[:, BW:2 * BW], AF.Tanh)
                nc.vector.tensor_tensor(h_all[r0:r1, :],
                                        sifo[r0:r1, 2 * BW:3 * BW],
                                        tct[r0:r1, :], ALU.mult)

            # --- emission: interleave phase-1 chunks with recurrence ---
            steps_done = 0
            for ci in range(NCHUNK):
                emit_phase1_chunk(ci)
                tmax = min(T, (ci + 1) * (CHUNK // BL))
                while steps_done < tmax:
                    for c in range(CH):
                        emit_step(steps_done, c)
                    steps_done += 1
            while steps_done < NSTEP:
                for c in range(CH):
                    emit_step(steps_done, c)
                steps_done += 1

            # --- FC + softmax (per chain) ---
            for c in range(CH):
                h_all, relu2 = h_alls[c], relu2s[c]
                p1 = pg_pool.tile([16, BW], f32, tag=f"pg{c}")
                nc.tensor.matmul(p1[:], w1[:], h_all[:], start=True, stop=True)
                nc.scalar.activation(relu2[0:16, :], p1[:], AF.Relu)
                p2 = pg_pool.tile([BW, C], f32, tag=f"pg{c}")
                nc.tensor.matmul(p2[:], relu2[:], w2[:], start=True, stop=True)
                negmax = work.tile([BW, 1], f32, tag=f"negmax{c}")
                nc.vector.reduce_max(negmax[:], p2[:], mybir.AxisListType.X,
                                     negate=True)
                esum = work.tile([BW, 1], f32, tag=f"esum{c}")
                evals = work.tile([BW, C], f32, tag=f"evals{c}")
                nc.scalar.activation(evals[:], p2[:], AF.Exp, bias=negmax[:],
                                     accum_out=esum[:])
                rinv = work.tile([BW, 1], f32, tag=f"rinv{c}")
                nc.vector.reciprocal(rinv[:], esum[:])
                prob = work.tile([BW, C], f32, tag=f"prob{c}")
                nc.vector.tensor_scalar(prob[:], evals[:], rinv[:], None,
                                        ALU.mult)
                nc.sync.dma_start(out=out_d[c * BW:(c + 1) * BW, :],
                                  in_=prob[:])

    nc.compile()
    return nc


def _prep_inputs(inputs, x_dtype):
    x = inputs["x"]
    consts = build_host_constants(inputs, x_dtype)
    xdt = _np_dt(x_dtype)
    in_maps = []
    for g in range(NCORES):
        xc = x[g * BL:(g + 1) * BL]                      # [32, 200, 1086]
        xr = xc.reshape(BL, THI, TL, I).transpose(3, 1, 0, 2)  # [I,25,32,8]
        xf = np.ascontiguousarray(xr).reshape(I, NCOLS).astype(xdt)
        m = dict(x=xf, wproj=consts["W_proj"], w1=consts["W1e"],
                 w2=consts["W2"], sel=consts["SEL"])
        for t in TYPES:
            m[f"lhs_{t}"] = consts[f"lhsT_{t}"]
        in_maps.append(m)
    return in_maps


def kernel(**inputs):
    from concourse.bass_utils import run_bass_kernel_spmd

    x_dtype = CFG["x_dtype"]
    key = ("nc", x_dtype, CFG["nchains"])
    if key not in _BUILD_CACHE:
        _BUILD_CACHE[key] = build_bass(x_dtype, CFG["nchains"])
    nc = _BUILD_CACHE[key]
    in_maps = _prep_inputs(inputs, x_dtype)
    res = run_bass_kernel_spmd(nc, in_maps, list(range(NCORES)))
    out = np.concatenate([res.results[g]["out"] for g in range(NCORES)], axis=0)
    return out.astype(np.float32)


# revision 23
# speedup vs baseline: 1.4700x; 1.0127x over previous
"""Trainium2 Bass kernel for a 4-layer LSTM classifier (H=16) over 8 NeuronCores.

Strategy: pure data parallel, batch 256 -> 32/core. Per core:
  phase 1: input projection pre0 = x @ W_ih_l0a^T streamed from HBM; host
           pre-transposes x so contraction dim I lands on SBUF partitions
           with fully contiguous DMA rows. Output is partition-regrouped
           via SBUF DMA into a [16, (type, b, tl)] layout.
  phase 2: wavefront recurrence over (layer, t): all 4 layers' gates via 4
           matmuls (one per gate type i/f/o/g) into one [64, 128] PSUM tile
           (col = 32*type + b). All elementwise ops are partition-aligned
           [64, *] ops; h writes land directly in h_all[0:64].
  phase 3: FC(16->16)+ReLU via select-folded matmul on h_all, FC(16->15),
           softmax, DMA out [32, 15] per core.
"""

import sys

if "/opt/trn_rl_repo" not in sys.path:
    sys.path.insert(0, "/opt/trn_rl_repo")

import numpy as np

# ---- problem constants (hardcoded per contract) ----
B, T, I, H, C = 256, 200, 1086, 16, 15
NCORES = 8
BL = B // NCORES          # 32 batch per core
TL = 8                    # t-interleave factor
THI = T // TL             # 25
NCOLS = BL * T            # 6400
CHUNK = 512               # phase-1 matmul free dim (= 2 t_hi blocks = 16 t)
NCHUNK = (NCOLS + CHUNK - 1) // CHUNK  # 13 (last = 256)
KCH = [128] * 8 + [62]    # 1086 contraction chunks
NSTEP = T + 3             # 203 wavefront steps

CFG = dict(
    x_dtype="bfloat16",    # or "float32"
    rec_dtype="bfloat16",  # recurrence state/gate dtype
    nchains=2,             # independent phase-offset recurrence chains
)

_BUILD_CACHE = {}


def _np_dt(name):
    import ml_dtypes
    return np.dtype(ml_dtypes.bfloat16) if name == "bfloat16" else np.dtype(name)


def _gate_rows(w):
    # torch gate row order in 4H matrices: i, f, g, o
    return dict(i=w[0:H], f=w[H:2 * H], g=w[2 * H:3 * H], o=w[3 * H:4 * H])


TYPES = ["i", "f", "o", "g"]  # gate-type order used everywhere on-chip


def build_host_constants(wd, x_dtype, rec_dtype="float32"):
    f32 = np.float32
    # phase-1 W: rows I, cols 64 = (type-major: i0,f0,o0,g0) x16 units
    g0 = _gate_rows(wd["w_ih_l0a"])
    W_proj = np.zeros((I, 64), f32)
    for j, t in enumerate(TYPES):
        W_proj[:, 16 * j:16 * j + 16] = g0[t].T
    W_proj = W_proj.astype(_np_dt(x_dtype))

    # recurrence weights: per gate type, lhsT [65, 64]
    # h_all rows: h0 0:16, h1 16:32, h2 32:48, h3 48:64, ONE 64
    # cols: unit m = 16*l + u
    hh = [_gate_rows(wd["w_hh_l0a"]), _gate_rows(wd["w_hh_l0b"]),
          _gate_rows(wd["w_hh_l1a"]), _gate_rows(wd["w_hh_l1b"])]
    ih = [None, _gate_rows(wd["w_ih_l0b"]), _gate_rows(wd["w_ih_l1a"]),
          _gate_rows(wd["w_ih_l1b"])]
    bb = [_gate_rows(wd["b_l0a"][:, None]), _gate_rows(wd["b_l0b"][:, None]),
          _gate_rows(wd["b_l1a"][:, None]), _gate_rows(wd["b_l1b"][:, None])]
    lhsT = {}
    for t in TYPES:
        M = np.zeros((65, 64), f32)
        for l in range(4):
            cs = slice(16 * l, 16 * l + 16)
            M[16 * l:16 * l + 16, cs] = hh[l][t].T      # recurrent h_l
            if l >= 1:
                M[16 * (l - 1):16 * l, cs] = ih[l][t].T  # input h_{l-1}
            M[64, cs] = bb[l][t][:, 0]                   # bias
        lhsT[t] = M

    # fc1 folded onto h_all: out1[u,b] = sum_k W1e[k,u] h_all[k,b]
    W1e = np.zeros((65, 16), f32)
    W1e[48:64] = wd["w_fc1"].T      # h3 rows
    W1e[64] = wd["b_fc1"]
    # relu2 tile is [33, BL]: rows 0:16 = relu(fc1), rows 16:32 = zeros,
    # row 32 = ones (32-aligned partition for the memset)
    W2 = np.zeros((33, 15), f32)
    W2[0:16] = wd["w_fc2"].T
    W2[32] = wd["b_fc2"]
    # pre-injection select: maps pre row u -> pg row u (l0 units), zeros rows 16:64
    SEL = np.zeros((16, 64), f32)
    SEL[np.arange(16), np.arange(16)] = 1.0
    rdt_np = _np_dt(rec_dtype)
    return dict(W_proj=W_proj, lhsT_i=lhsT["i"].astype(rdt_np),
                lhsT_f=lhsT["f"].astype(rdt_np), lhsT_o=lhsT["o"].astype(rdt_np),
                lhsT_g=lhsT["g"].astype(rdt_np), W1e=W1e.astype(rdt_np),
                W2=W2, SEL=SEL)


def build_bass(x_dtype="float32", nchains=2, rec_dtype="float32"):
    from concourse import bacc, mybir

    from concourse.tile import TileContext

    dt = mybir.dt
    xdt = dt.bfloat16 if x_dtype == "bfloat16" else dt.float32
    f32 = dt.float32
    rdt = dt.bfloat16 if rec_dtype == "bfloat16" else dt.float32
    AF = mybir.ActivationFunctionType
    ALU = mybir.AluOpType

    nc = bacc.Bacc("TRN2", target_bir_lowering=False, debug=False,
                   num_devices=NCORES)

    xin = nc.dram_tensor("x", [I, NCOLS], xdt, kind="ExternalInput").ap()
    wproj_d = nc.dram_tensor("wproj", [I, 64], xdt, kind="ExternalInput").ap()
    lhs_d = {t: nc.dram_tensor(f"lhs_{t}", [65, 64], rdt,
                               kind="ExternalInput").ap() for t in TYPES}
    w1_d = nc.dram_tensor("w1", [65, 16], rdt, kind="ExternalInput").ap()
    w2_d = nc.dram_tensor("w2", [33, 15], f32, kind="ExternalInput").ap()
    sel_d = nc.dram_tensor("sel", [16, 64], f32, kind="ExternalInput").ap()
    out_d = nc.dram_tensor("out", [BL, C], f32, kind="ExternalOutput").ap()

    with TileContext(nc) as tc:
        import contextlib
        with contextlib.ExitStack() as ctx:
            wpool = ctx.enter_context(tc.tile_pool(name="weights", bufs=9))
            xpool = ctx.enter_context(tc.tile_pool(name="xtiles", bufs=4))
            prepool = ctx.enter_context(tc.tile_pool(name="pre", bufs=THI))
            state = ctx.enter_context(tc.tile_pool(name="state", bufs=1))
            work = ctx.enter_context(tc.tile_pool(name="work", bufs=3))
            pg_pool = ctx.enter_context(
                tc.tile_pool(name="pgates", bufs=2, space="PSUM"))
            pgg_pool = ctx.enter_context(
                tc.tile_pool(name="pgg", bufs=1, space="PSUM"))
            px_pool = ctx.enter_context(
                tc.tile_pool(name="pproj", bufs=2, space="PSUM"))

            # --- weights ---
            wproj_t = []
            k0 = 0
            for kk in KCH:
                wt = wpool.tile([128, 64], xdt, tag="wproj")
                nc.sync.dma_start(out=wt[0:kk, :], in_=wproj_d[k0:k0 + kk, :])
                wproj_t.append(wt)
                k0 += kk
            lhs = {}
            for t in TYPES:
                lt = wpool.tile([65, 64], rdt, tag=f"lhs_{t}")
                nc.sync.dma_start(out=lt[:], in_=lhs_d[t][:])
                lhs[t] = lt
            w1 = wpool.tile([65, 16], rdt, tag="w1")
            nc.sync.dma_start(out=w1[:], in_=w1_d[:])
            w2 = wpool.tile([33, 15], f32, tag="w2")
            nc.sync.dma_start(out=w2[:], in_=w2_d[:])
            sel = wpool.tile([16, 64], f32, tag="sel")
            nc.sync.dma_start(out=sel[:], in_=sel_d[:])

            # --- persistent state (one set per chain) ---
            CH = nchains
            BW = BL // CH
            h_alls, gcts, relu2s = [], [], []
            for c in range(CH):
                h_all = state.tile([65, BW], rdt, tag=f"h_all{c}")
                nc.vector.memset(h_all[:], 0.0)
                nc.vector.memset(h_all[64:65, :], 1.0)
                # gct: cols 0:BW = tanh(g_raw), cols BW:2BW = c (persistent)
                gct = state.tile([64, 2 * BW], rdt, tag=f"gct{c}")
                nc.vector.memset(gct[:], 0.0)
                relu2 = state.tile([33, BW], f32, tag=f"relu2{c}")
                nc.vector.memset(relu2[:], 0.0)
                nc.vector.memset(relu2[32:33, :], 1.0)
                h_alls.append(h_all)
                gcts.append(gct)
                relu2s.append(relu2)

            pre_tiles = [None] * THI

            def emit_phase1_chunk(ci):
                c0 = ci * CHUNK
                cw = min(CHUNK, NCOLS - c0)
                px = px_pool.tile([64, CHUNK], f32, tag="px")
                k0 = 0
                for ki, kk in enumerate(KCH):
                    xt = xpool.tile([128, CHUNK], xdt, tag="xt")
                    nc.sync.dma_start(out=xt[0:kk, 0:cw],
                                      in_=xin[k0:k0 + kk, c0:c0 + cw])
                    nc.tensor.matmul(px[:, 0:cw], wproj_t[ki][0:kk, :],
                                     xt[0:kk, 0:cw],
                                     start=(ki == 0), stop=(ki == len(KCH) - 1))
                    k0 += kk
                # stage psum -> SBUF, then partition-regroup into pre tiles
                # [16, (type, b, tl)] via SBUF->SBUF DMA
                stage = xpool.tile([64, CHUNK], f32, tag="stage")
                nc.scalar.copy(stage[:, 0:cw], px[:, 0:cw])
                nblk = cw // (BL * TL)
                for bi in range(nblk):
                    th = (c0 // (BL * TL)) + bi
                    pt = prepool.tile([16, 4, BL, TL], f32, tag="pre")
                    for j in range(4):
                        src = stage[16 * j:16 * j + 16,
                                    bi * BL * TL:(bi + 1) * BL * TL]
                        nc.sync.dma_start(out=pt[:, j, :, :], in_=src)
                    pre_tiles[th] = pt

            def emit_step(s, c):
                h_all, gct = h_alls[c], gcts[c]
                lmin = max(0, s - (T - 1))
                lmax = min(3, s)
                # write range for state updates; starts must be 32-aligned,
                # so widen r0 down (clobbered rows are only read by inactive
                # layers afterwards -- harmless garbage)
                r0 = (16 * lmin // 32) * 32
                r1 = 16 * (lmax + 1)
                # g gates in their own psum tile/accum-group so TANHG can
                # start right after mm_g, hiding under the i/f/o matmuls
                pg = pg_pool.tile([64, 3 * BW], f32, tag=f"pg{c}")
                pgg = pgg_pool.tile([64, BW], f32, tag=f"pgg{c}")
                has_pre = s < T
                if has_pre:
                    th, tl = s // TL, s % TL
                    pslice = pre_tiles[th][:, :, c * BW:(c + 1) * BW, tl]
                    nc.tensor.matmul(pgg[:], sel[:], pslice[:, 3, :],
                                     start=True, stop=False,
                                     skip_group_check=True)
                    nc.tensor.matmul(pg[:], sel[:], pslice[:, 0:3, :],
                                     start=True, stop=False,
                                     skip_group_check=True)
                nc.tensor.matmul(pgg[:], lhs["g"][:], h_all[:],
                                 start=not has_pre, stop=True,
                                 skip_group_check=True)
                nc.scalar.activation(gct[:, 0:BW], pgg[:], AF.Tanh)
                for j, t in enumerate(TYPES[:3]):
                    nc.tensor.matmul(pg[:, BW * j:BW * (j + 1)], lhs[t][:],
                                     h_all[:], start=not has_pre, stop=True,
                                     skip_group_check=True)
                sifo = work.tile([64, 3 * BW], rdt, tag=f"sifo{c}")
                nc.scalar.activation(sifo[:], pg[:], AF.Sigmoid)
                tmp = work.tile([64, 2 * BW], rdt, tag=f"tmp{c}")
                nc.vector.tensor_tensor(tmp[:], sifo[:, 0:2 * BW], gct[:],
                                        ALU.mult)
                nc.vector.tensor_tensor(gct[r0:r1, BW:2 * BW],
                                        tmp[r0:r1, 0:BW],
                                        tmp[r0:r1, BW:2 * BW], ALU.add)
                tct = work.tile([64, BW], rdt, tag=f"tct{c}")
                nc.scalar.activation(tct[:], gct[:, BW:2 * BW], AF.Tanh)
                nc.vector.tensor_tensor(h_all[r0:r1, :],
                                        sifo[r0:r1, 2 * BW:3 * BW],
                                        tct[r0:r1, :], ALU.mult)

            # --- emission: interleave phase-1 chunks with recurrence ---
            steps_done = 0
            for ci in range(NCHUNK):
                emit_phase1_chunk(ci)
                tmax = min(T, (ci + 1) * (CHUNK // BL))
                while steps_done < tmax:
                    for c in range(CH):
                        emit_step(steps_done, c)
                    steps_done += 1
            while steps_done < NSTEP:
                for c in range(CH):
                    emit_step(steps_done, c)
                steps_done += 1

            # --- FC + softmax (per chain) ---
            for c in range(CH):
                h_all, relu2 = h_alls[c], relu2s[c]
                p1 = pg_pool.tile([16, BW], f32, tag=f"pg{c}")
                nc.tensor.matmul(p1[:], w1[:], h_all[:], start=True, stop=True)
                nc.scalar.activation(relu2[0:16, :], p1[:], AF.Relu)
                p2 = pg_pool.tile([BW, C], f32, tag=f"pg{c}")
                nc.tensor.matmul(p2[:], relu2[:], w2[:], start=True, stop=True)
                negmax = work.tile([BW, 1], f32, tag=f"negmax{c}")
                nc.vector.reduce_max(negmax[:], p2[:], mybir.AxisListType.X,
                                     negate=True)
                esum = work.tile([BW, 1], f32, tag=f"esum{c}")
                evals = work.tile([BW, C], f32, tag=f"evals{c}")
                nc.scalar.activation(evals[:], p2[:], AF.Exp, bias=negmax[:],
                                     accum_out=esum[:])
                rinv = work.tile([BW, 1], f32, tag=f"rinv{c}")
                nc.vector.reciprocal(rinv[:], esum[:])
                prob = work.tile([BW, C], f32, tag=f"prob{c}")
                nc.vector.tensor_scalar(prob[:], evals[:], rinv[:], None,
                                        ALU.mult)
                nc.sync.dma_start(out=out_d[c * BW:(c + 1) * BW, :],
                                  in_=prob[:])

    nc.compile()
    return nc


def _prep_inputs(inputs, x_dtype):
    x = inputs["x"]
    consts = build_host_constants(inputs, x_dtype, CFG["rec_dtype"])
    xdt = _np_dt(x_dtype)
    in_maps = []
    for g in range(NCORES):
        xc = x[g * BL:(g + 1) * BL]                      # [32, 200, 1086]
        xr = xc.reshape(BL, THI, TL, I).transpose(3, 1, 0, 2)  # [I,25,32,8]
        xf = np.ascontiguousarray(xr).reshape(I, NCOLS).astype(xdt)
        m = dict(x=xf, wproj=consts["W_proj"], w1=consts["W1e"],
                 w2=consts["W2"], sel=consts["SEL"])
        for t in TYPES:
            m[f"lhs_{t}"] = consts[f"lhsT_{t}"]
        in_maps.append(m)
    return in_maps


def kernel(**inputs):
    from concourse.bass_utils import run_bass_kernel_spmd

    x_dtype = CFG["x_dtype"]
    key = ("nc", x_dtype, CFG["nchains"], CFG["rec_dtype"])
    if key not in _BUILD_CACHE:
        _BUILD_CACHE[key] = build_bass(x_dtype, CFG["nchains"], CFG["rec_dtype"])
    nc = _BUILD_CACHE[key]
    in_maps = _prep_inputs(inputs, x_dtype)
    res = run_bass_kernel_spmd(nc, in_maps, list(range(NCORES)))
    out = np.concatenate([res.results[g]["out"] for g in range(NCORES)], axis=0)
    return out.astype(np.float32)


# revision 25
# speedup vs baseline: 1.4760x; 1.0041x over previous
"""Trainium2 Bass kernel for a 4-layer LSTM classifier (H=16) over 8 NeuronCores.

Strategy: pure data parallel, batch 256 -> 32/core. Per core:
  phase 1: input projection pre0 = x @ W_ih_l0a^T streamed from HBM; host
           pre-transposes x so contraction dim I lands on SBUF partitions
           with fully contiguous DMA rows. Output is partition-regrouped
           via SBUF DMA into a [16, (type, b, tl)] layout.
  phase 2: wavefront recurrence over (layer, t): all 4 layers' gates via 4
           matmuls (one per gate type i/f/o/g) into one [64, 128] PSUM tile
           (col = 32*type + b). All elementwise ops are partition-aligned
           [64, *] ops; h writes land directly in h_all[0:64].
  phase 3: FC(16->16)+ReLU via select-folded matmul on h_all, FC(16->15),
           softmax, DMA out [32, 15] per core.
"""

import sys

if "/opt/trn_rl_repo" not in sys.path:
    sys.path.insert(0, "/opt/trn_rl_repo")

import numpy as np

# ---- problem constants (hardcoded per contract) ----
B, T, I, H, C = 256, 200, 1086, 16, 15
NCORES = 8
BL = B // NCORES          # 32 batch per core
TL = 8                    # t-interleave factor
THI = T // TL             # 25
NCOLS = BL * T            # 6400
CHUNK = 512               # phase-1 matmul free dim (= 2 t_hi blocks = 16 t)
NCHUNK = (NCOLS + CHUNK - 1) // CHUNK  # 13 (last = 256)
KCH = [128] * 8 + [62]    # 1086 contraction chunks
NSTEP = T + 3             # 203 wavefront steps

CFG = dict(
    x_dtype="bfloat16",    # or "float32"
    rec_dtype="bfloat16",  # recurrence state/gate dtype
    nchains=2,             # independent phase-offset recurrence chains
)

_BUILD_CACHE = {}


def _np_dt(name):
    import ml_dtypes
    return np.dtype(ml_dtypes.bfloat16) if name == "bfloat16" else np.dtype(name)


def _gate_rows(w):
    # torch gate row order in 4H matrices: i, f, g, o
    return dict(i=w[0:H], f=w[H:2 * H], g=w[2 * H:3 * H], o=w[3 * H:4 * H])


TYPES = ["i", "f", "o", "g"]  # gate-type order used everywhere on-chip


def build_host_constants(wd, x_dtype, rec_dtype="float32"):
    f32 = np.float32
    # phase-1 W: rows I, cols 64 = (type-major: i0,f0,o0,g0) x16 units
    g0 = _gate_rows(wd["w_ih_l0a"])
    W_proj = np.zeros((I, 64), f32)
    for j, t in enumerate(TYPES):
        W_proj[:, 16 * j:16 * j + 16] = g0[t].T
    W_proj = W_proj.astype(_np_dt(x_dtype))

    # recurrence weights: per gate type, lhsT [65, 64]
    # h_all rows: h0 0:16, h1 16:32, h2 32:48, h3 48:64, ONE 64
    # cols: unit m = 16*l + u
    hh = [_gate_rows(wd["w_hh_l0a"]), _gate_rows(wd["w_hh_l0b"]),
          _gate_rows(wd["w_hh_l1a"]), _gate_rows(wd["w_hh_l1b"])]
    ih = [None, _gate_rows(wd["w_ih_l0b"]), _gate_rows(wd["w_ih_l1a"]),
          _gate_rows(wd["w_ih_l1b"])]
    bb = [_gate_rows(wd["b_l0a"][:, None]), _gate_rows(wd["b_l0b"][:, None]),
          _gate_rows(wd["b_l1a"][:, None]), _gate_rows(wd["b_l1b"][:, None])]
    lhsT = {}
    for t in TYPES:
        M = np.zeros((65, 64), f32)
        for l in range(4):
            cs = slice(16 * l, 16 * l + 16)
            M[16 * l:16 * l + 16, cs] = hh[l][t].T      # recurrent h_l
            if l >= 1:
                M[16 * (l - 1):16 * l, cs] = ih[l][t].T  # input h_{l-1}
            M[64, cs] = bb[l][t][:, 0]                   # bias
        lhsT[t] = M

    # fc1 folded onto h_all: out1[u,b] = sum_k W1e[k,u] h_all[k,b]
    W1e = np.zeros((65, 16), f32)
    W1e[48:64] = wd["w_fc1"].T      # h3 rows
    W1e[64] = wd["b_fc1"]
    # relu2 tile is [33, BL]: rows 0:16 = relu(fc1), rows 16:32 = zeros,
    # row 32 = ones (32-aligned partition for the memset)
    W2 = np.zeros((33, 15), f32)
    W2[0:16] = wd["w_fc2"].T
    W2[32] = wd["b_fc2"]
    # pre-injection select: maps pre row u -> pg row u (l0 units), zeros rows 16:64
    SEL = np.zeros((16, 64), f32)
    SEL[np.arange(16), np.arange(16)] = 1.0
    SEL = SEL.astype(_np_dt(x_dtype))
    rdt_np = _np_dt(rec_dtype)
    return dict(W_proj=W_proj, lhsT_i=lhsT["i"].astype(rdt_np),
                lhsT_f=lhsT["f"].astype(rdt_np), lhsT_o=lhsT["o"].astype(rdt_np),
                lhsT_g=lhsT["g"].astype(rdt_np), W1e=W1e.astype(rdt_np),
                W2=W2, SEL=SEL)


def build_bass(x_dtype="float32", nchains=2, rec_dtype="float32"):
    from concourse import bacc, mybir

    from concourse.tile import TileContext

    dt = mybir.dt
    xdt = dt.bfloat16 if x_dtype == "bfloat16" else dt.float32
    f32 = dt.float32
    rdt = dt.bfloat16 if rec_dtype == "bfloat16" else dt.float32
    AF = mybir.ActivationFunctionType
    ALU = mybir.AluOpType

    nc = bacc.Bacc("TRN2", target_bir_lowering=False, debug=False,
                   num_devices=NCORES)

    xin = nc.dram_tensor("x", [I, NCOLS], xdt, kind="ExternalInput").ap()
    wproj_d = nc.dram_tensor("wproj", [I, 64], xdt, kind="ExternalInput").ap()
    lhs_d = {t: nc.dram_tensor(f"lhs_{t}", [65, 64], rdt,
                               kind="ExternalInput").ap() for t in TYPES}
    w1_d = nc.dram_tensor("w1", [65, 16], rdt, kind="ExternalInput").ap()
    w2_d = nc.dram_tensor("w2", [33, 15], f32, kind="ExternalInput").ap()
    sel_d = nc.dram_tensor("sel", [16, 64], xdt, kind="ExternalInput").ap()
    out_d = nc.dram_tensor("out", [BL, C], f32, kind="ExternalOutput").ap()

    with TileContext(nc) as tc:
        import contextlib
        with contextlib.ExitStack() as ctx:
            wpool = ctx.enter_context(tc.tile_pool(name="weights", bufs=9))
            xpool = ctx.enter_context(tc.tile_pool(name="xtiles", bufs=4))
            prepool = ctx.enter_context(tc.tile_pool(name="pre", bufs=THI))
            state = ctx.enter_context(tc.tile_pool(name="state", bufs=1))
            work = ctx.enter_context(tc.tile_pool(name="work", bufs=3))
            pg_pool = ctx.enter_context(
                tc.tile_pool(name="pgates", bufs=2, space="PSUM"))
            pgg_pool = ctx.enter_context(
                tc.tile_pool(name="pgg", bufs=1, space="PSUM"))
            px_pool = ctx.enter_context(
                tc.tile_pool(name="pproj", bufs=2, space="PSUM"))

            # --- weights ---
            wproj_t = []
            k0 = 0
            for kk in KCH:
                wt = wpool.tile([128, 64], xdt, tag="wproj")
                nc.sync.dma_start(out=wt[0:kk, :], in_=wproj_d[k0:k0 + kk, :])
                wproj_t.append(wt)
                k0 += kk
            lhs = {}
            for t in TYPES:
                lt = wpool.tile([65, 64], rdt, tag=f"lhs_{t}")
                nc.sync.dma_start(out=lt[:], in_=lhs_d[t][:])
                lhs[t] = lt
            w1 = wpool.tile([65, 16], rdt, tag="w1")
            nc.sync.dma_start(out=w1[:], in_=w1_d[:])
            w2 = wpool.tile([33, 15], f32, tag="w2")
            nc.sync.dma_start(out=w2[:], in_=w2_d[:])
            sel = wpool.tile([16, 64], xdt, tag="sel")
            nc.sync.dma_start(out=sel[:], in_=sel_d[:])

            # --- persistent state (one set per chain) ---
            CH = nchains
            BW = BL // CH
            h_alls, gcts, relu2s = [], [], []
            for c in range(CH):
                h_all = state.tile([65, BW], rdt, tag=f"h_all{c}")
                nc.vector.memset(h_all[:], 0.0)
                nc.vector.memset(h_all[64:65, :], 1.0)
                # gct: cols 0:BW = tanh(g_raw), cols BW:2BW = c (persistent)
                gct = state.tile([64, 2 * BW], rdt, tag=f"gct{c}")
                nc.vector.memset(gct[:], 0.0)
                relu2 = state.tile([33, BW], f32, tag=f"relu2{c}")
                nc.vector.memset(relu2[:], 0.0)
                nc.vector.memset(relu2[32:33, :], 1.0)
                h_alls.append(h_all)
                gcts.append(gct)
                relu2s.append(relu2)

            pre_tiles = [None] * THI

            def emit_phase1_chunk(c0, cw):
                px = px_pool.tile([64, CHUNK], f32, tag="px")
                k0 = 0
                for ki, kk in enumerate(KCH):
                    xt = xpool.tile([128, CHUNK], xdt, tag="xt")
                    nc.sync.dma_start(out=xt[0:kk, 0:cw],
                                      in_=xin[k0:k0 + kk, c0:c0 + cw])
                    nc.tensor.matmul(px[:, 0:cw], wproj_t[ki][0:kk, :],
                                     xt[0:kk, 0:cw],
                                     start=(ki == 0), stop=(ki == len(KCH) - 1))
                    k0 += kk
                # stage psum -> SBUF, then partition-regroup into pre tiles
                # [16, (type, b, tl)] via SBUF->SBUF DMA
                stage = xpool.tile([64, CHUNK], xdt, tag="stage")
                nc.vector.tensor_copy(stage[:, 0:cw], px[:, 0:cw])
                nblk = cw // (BL * TL)
                for bi in range(nblk):
                    th = (c0 // (BL * TL)) + bi
                    pt = prepool.tile([16, 4, BL, TL], xdt, tag="pre")
                    for j in range(4):
                        src = stage[16 * j:16 * j + 16,
                                    bi * BL * TL:(bi + 1) * BL * TL]
                        nc.sync.dma_start(out=pt[:, j, :, :], in_=src)
                    pre_tiles[th] = pt

            def emit_step(s, c):
                h_all, gct = h_alls[c], gcts[c]
                lmin = max(0, s - (T - 1))
                lmax = min(3, s)
                # write range for state updates; starts must be 32-aligned,
                # so widen r0 down (clobbered rows are only read by inactive
                # layers afterwards -- harmless garbage)
                r0 = (16 * lmin // 32) * 32
                r1 = 16 * (lmax + 1)
                # g gates in their own psum tile/accum-group so TANHG can
                # start right after mm_g, hiding under the i/f/o matmuls
                pg = pg_pool.tile([64, 3 * BW], f32, tag=f"pg{c}")
                pgg = pgg_pool.tile([64, BW], f32, tag=f"pgg{c}")
                has_pre = s < T
                if has_pre:
                    th, tl = s // TL, s % TL
                    pslice = pre_tiles[th][:, :, c * BW:(c + 1) * BW, tl]
                    nc.tensor.matmul(pgg[:], sel[:], pslice[:, 3, :],
                                     start=True, stop=False,
                                     skip_group_check=True)
                    nc.tensor.matmul(pg[:], sel[:], pslice[:, 0:3, :],
                                     start=True, stop=False,
                                     skip_group_check=True)
                nc.tensor.matmul(pgg[:], lhs["g"][:], h_all[:],
                                 start=not has_pre, stop=True,
                                 skip_group_check=True)
                nc.scalar.activation(gct[:, 0:BW], pgg[:], AF.Tanh)
                for j, t in enumerate(TYPES[:3]):
                    nc.tensor.matmul(pg[:, BW * j:BW * (j + 1)], lhs[t][:],
                                     h_all[:], start=not has_pre, stop=True,
                                     skip_group_check=True)
                sifo = work.tile([64, 3 * BW], rdt, tag=f"sifo{c}")
                nc.scalar.activation(sifo[:], pg[:], AF.Sigmoid)
                tmp = work.tile([64, 2 * BW], rdt, tag=f"tmp{c}")
                nc.vector.tensor_tensor(tmp[:], sifo[:, 0:2 * BW], gct[:],
                                        ALU.mult)
                nc.vector.tensor_tensor(gct[r0:r1, BW:2 * BW],
                                        tmp[r0:r1, 0:BW],
                                        tmp[r0:r1, BW:2 * BW], ALU.add)
                tct = work.tile([64, BW], rdt, tag=f"tct{c}")
                nc.scalar.activation(tct[:], gct[:, BW:2 * BW], AF.Tanh)
                nc.vector.tensor_tensor(h_all[r0:r1, :],
                                        sifo[r0:r1, 2 * BW:3 * BW],
                                        tct[r0:r1, :], ALU.mult)

            # --- emission: interleave phase-1 chunks with recurrence ---
            # first chunks are small so the recurrence starts early
            bounds, c0 = [], 0
            for cw in [256, 256] + [CHUNK] * NCHUNK:
                cw = min(cw, NCOLS - c0)
                if cw <= 0:
                    break
                bounds.append((c0, cw))
                c0 += cw
            steps_done = 0
            for c0, cw in bounds:
                emit_phase1_chunk(c0, cw)
                tmax = min(T, (c0 + cw) // BL)
                while steps_done < tmax:
                    for c in range(CH):
                        emit_step(steps_done, c)
                    steps_done += 1
            while steps_done < NSTEP:
                for c in range(CH):
                    emit_step(steps_done, c)
                steps_done += 1

            # --- FC + softmax (per chain) ---
            for c in range(CH):
                h_all, relu2 = h_alls[c], relu2s[c]
                p1 = pg_pool.tile([16, BW], f32, tag=f"pg{c}")
                nc.tensor.matmul(p1[:], w1[:], h_all[:], start=True, stop=True)
                nc.scalar.activation(relu2[0:16, :], p1[:], AF.Relu)
                p2 = pg_pool.tile([BW, C], f32, tag=f"pg{c}")
                nc.tensor.matmul(p2[:], relu2[:], w2[:], start=True, stop=True)
                negmax = work.tile([BW, 1], f32, tag=f"negmax{c}")
                nc.vector.reduce_max(negmax[:], p2[:], mybir.AxisListType.X,
                                     negate=True)
                esum = work.tile([BW, 1], f32, tag=f"esum{c}")
                evals = work.tile([BW, C], f32, tag=f"evals{c}")
                nc.scalar.activation(evals[:], p2[:], AF.Exp, bias=negmax[:],
                                     accum_out=esum[:])
                rinv = work.tile([BW, 1], f32, tag=f"rinv{c}")
                nc.vector.reciprocal(rinv[:], esum[:])
                prob = work.tile([BW, C], f32, tag=f"prob{c}")
                nc.vector.tensor_scalar(prob[:], evals[:], rinv[:], None,
                                        ALU.mult)
                nc.sync.dma_start(out=out_d[c * BW:(c + 1) * BW, :],
                                  in_=prob[:])

    nc.compile()
    return nc


def _prep_inputs(inputs, x_dtype):
    x = inputs["x"]
    consts = build_host_constants(inputs, x_dtype, CFG["rec_dtype"])
    xdt = _np_dt(x_dtype)
    in_maps = []
    for g in range(NCORES):
        xc = x[g * BL:(g + 1) * BL]                      # [32, 200, 1086]
        xr = xc.reshape(BL, THI, TL, I).transpose(3, 1, 0, 2)  # [I,25,32,8]
        xf = np.ascontiguousarray(xr).reshape(I, NCOLS).astype(xdt)
        m = dict(x=xf, wproj=consts["W_proj"], w1=consts["W1e"],
                 w2=consts["W2"], sel=consts["SEL"])
        for t in TYPES:
            m[f"lhs_{t}"] = consts[f"lhsT_{t}"]
        in_maps.append(m)
    return in_maps


def kernel(**inputs):
    from concourse.bass_utils import run_bass_kernel_spmd

    x_dtype = CFG["x_dtype"]
    key = ("nc", x_dtype, CFG["nchains"], CFG["rec_dtype"])
    if key not in _BUILD_CACHE:
        _BUILD_CACHE[key] = build_bass(x_dtype, CFG["nchains"], CFG["rec_dtype"])
    nc = _BUILD_CACHE[key]
    in_maps = _prep_inputs(inputs, x_dtype)
    res = run_bass_kernel_spmd(nc, in_maps, list(range(NCORES)))
    out = np.concatenate([res.results[g]["out"] for g in range(NCORES)], axis=0)
    return out.astype(np.float32)
